# revision 1
# baseline (speedup 1.0000x reference)
# PRoPE attention Trainium2 kernel (v2).
# Sharding: 8 cores = 2 batches x 4 head-groups (4 heads each).
# Per core, heads are processed as 2 pairs (pt). For each pair:
#   qT,kT = (128,L) projections with PRoPE applied as
#     e1 = C*acc, e2 = S2*acc (DVE), cc = camA@e1 + camB@e2 (PE),
#   where camB is the pair-swap permutation folded into the camera matrix
#   (replaces the stream_shuffle + add of v1).
#   v' is produced directly in (token, dim) layout by right-multiplying:
#     vo[tok,j] = sum_d e1v[d,tok]*camk[d,j] + e2v[d,tok]*camk2[d,j]
#   (replaces v1's PE transposes).
#   Attention: kT-stationary score matmuls (row-packed head pairs via
#   tile_position), exp on ScalarE (scale=1/8 fused), attn@v with a
#   ones-column in v for the softmax denominator (M=65), reciprocal via
#   DRAM bounce, rd folded into the oc copy, Mo via camo/camo2 pairs,
#   then the output projection streamed per token tile with immediate DMA.
# Host: packs weights/x into single-DMA layouts, builds cos/sin + the six
# camera matrix variants, and sums the 4 partial y's per batch.
import functools

import numpy as np

B, L, DM = 2, 2048, 1024
H, DH, NG = 16, 64, 16          # heads, head_dim, groups of 4
CAMS, PER_CAM = 8, 256
PX, PY, IW, IH, ROPE_BASE = 16, 16, 256.0, 256.0, 10000.0
HPG = 4                          # heads per group (per core)
HD4 = HPG * DH                   # 256 cols of qkv per core
N_CORES = 8


def _rope_tables():
    """C and S2 tables in (Dh, PER_CAM) layout, tiled to (128, L).
    S2 is the row-pair-swapped S~ table so that
    C*t + S~*swap(t) == C*t + swap(S2*t)."""
    idx = np.arange(PER_CAM)
    u = ((idx % PX) + 0.5) * (IW / PX)
    v = ((idx // PX) + 0.5) * (IH / PY)
    freqs = (np.float32(ROPE_BASE) ** (-(np.arange(NG, dtype=np.float32)) / np.float32(NG)))
    tu = (u[:, None] * freqs[None, :]).astype(np.float32)   # (P, G)
    tv = (v[:, None] * freqs[None, :]).astype(np.float32)
    ca, sa, cb, sb = np.cos(tu), np.sin(tu), np.cos(tv), np.sin(tv)
    Cq = np.zeros((DH, PER_CAM), np.float32)
    Sq = np.zeros((DH, PER_CAM), np.float32)
    for g in range(NG):
        Cq[4 * g + 0] = ca[:, g]; Cq[4 * g + 1] = ca[:, g]
        Cq[4 * g + 2] = cb[:, g]; Cq[4 * g + 3] = cb[:, g]
        # S~ rows: [-sa, sa, -sb, sb]
        Sq[4 * g + 0] = -sa[:, g]; Sq[4 * g + 1] = sa[:, g]
        Sq[4 * g + 2] = -sb[:, g]; Sq[4 * g + 3] = sb[:, g]
    # S2[p] = S~[p^1]
    S2 = np.zeros_like(Sq)
    for p in range(DH):
        S2[p] = Sq[p ^ 1]
    CqL = np.tile(Cq, (1, CAMS))           # (64, 2048)
    S2L = np.tile(S2, (1, CAMS))
    SqL = np.tile(Sq, (1, CAMS))
    csc = np.tile(CqL, (2, 1))             # (128, 2048)
    css2 = np.tile(S2L, (2, 1))
    css = np.tile(SqL, (2, 1))
    return csc, css2, css


def _cam_mats(viewmats, Ks):
    K4 = np.zeros((B, CAMS, 4, 4), np.float32)
    K4[..., :3, :3] = Ks
    K4[..., 3, 3] = 1.0
    P = (K4 @ viewmats).astype(np.float32)
    P_inv = np.linalg.inv(P.astype(np.float64)).astype(np.float32)
    return P, P_inv


@functools.lru_cache(maxsize=1)
def _build_nc():
    import concourse.bass as bass
    import concourse.mybir as mybir
    from concourse.tile import TileContext
    from contextlib import ExitStack

    dt = mybir.dt
    f32 = dt.float32
    f32r = dt.float32r
    bf16 = dt.bfloat16
    ALU = mybir.AluOpType
    ACT = mybir.ActivationFunctionType

    nc = bass.Bass("TRN2", target_bir_lowering=False, debug=False,
                   num_devices=N_CORES)

    # packed inputs (one DMA each unless split for pipelining)
    xt_d = nc.dram_tensor("xt", [128, 8 * L], bf16, kind="ExternalInput")
    wq_d = nc.dram_tensor("wq", [128, 8 * 256], bf16, kind="ExternalInput")
    wk_d = nc.dram_tensor("wk", [128, 8 * 256], bf16, kind="ExternalInput")
    wv_d = nc.dram_tensor("wv", [128, 8 * 256], bf16, kind="ExternalInput")
    wp_d = nc.dram_tensor("wp", [128, 2 * DM], bf16, kind="ExternalInput")
    csc_d = nc.dram_tensor("csc", [128, L], f32, kind="ExternalInput")
    css2_d = nc.dram_tensor("css2", [128, L], f32, kind="ExternalInput")
    css_d = nc.dram_tensor("css", [128, L], f32, kind="ExternalInput")
    camq_d = nc.dram_tensor("camq", [128, 8 * 128], f32, kind="ExternalInput")
    camq2_d = nc.dram_tensor("camq2", [128, 8 * 128], f32, kind="ExternalInput")
    camk_d = nc.dram_tensor("camk", [128, 8 * 128], f32, kind="ExternalInput")
    camk2_d = nc.dram_tensor("camk2", [128, 8 * 128], f32, kind="ExternalInput")
    camo_d = nc.dram_tensor("camo", [128, 8 * 128], f32, kind="ExternalInput")
    camo2_d = nc.dram_tensor("camo2", [128, 8 * 128], f32, kind="ExternalInput")
    ones_d = nc.dram_tensor("ones", [128, 4], bf16, kind="ExternalInput")
    yp_d = nc.dram_tensor("yp", [L, DM], f32, kind="ExternalOutput")
    bounce_d = nc.dram_tensor("bounce", [16, 512], f32, kind="Internal")
    bounce2_d = nc.dram_tensor("bounce2", [16, 512], f32, kind="Internal")

    with TileContext(nc) as tc, ExitStack() as ctx:
        # ---- persistent pools --------------------------------------------
        ptab = ctx.enter_context(tc.tile_pool(name="ptab", bufs=4))
        pcam = ctx.enter_context(tc.tile_pool(name="pcam", bufs=6))
        pw = ctx.enter_context(tc.tile_pool(name="pw", bufs=5))
        px = ctx.enter_context(tc.tile_pool(name="px", bufs=4))
        pqk = ctx.enter_context(tc.tile_pool(name="pqk", bufs=4))
        pvt = ctx.enter_context(tc.tile_pool(name="pvt", bufs=32))
        popt = ctx.enter_context(tc.tile_pool(name="popt", bufs=8))
        py = ctx.enter_context(tc.tile_pool(name="py", bufs=4))

        # weights + x first (critical path to first matmul), on sync queue
        wq = pw.tile([128, 8 * 256], bf16, tag="w", bufs=5, name="wq")
        wk = pw.tile([128, 8 * 256], bf16, tag="w", bufs=5, name="wk")
        wv = pw.tile([128, 8 * 256], bf16, tag="w", bufs=5, name="wv")
        wp = pw.tile([128, 2 * DM], bf16, tag="w", bufs=5, name="wp")
        xt = px.tile([128, 8 * L], bf16, tag="xt", bufs=1, name="xt")
        xcol = lambda d, lb: 4096 * lb + 512 * d
        nc.sync.dma_start(wq[:], wq_d[:, :])
        nc.sync.dma_start(xt[:, 0:4096], xt_d[:, 0:4096])
        nc.sync.dma_start(wk[:], wk_d[:, :])
        nc.sync.dma_start(wv[:], wv_d[:, :])
        # x in lb-major quarters: quarter lb = all 8 d-chunks of 512 tokens,
        # so each projection block's full contraction arrives together
        for qtr in range(1, 4):
            s = slice(4096 * qtr, 4096 * qtr + 4096)
            nc.sync.dma_start(xt[:, s], xt_d[:, s])
        nc.scalar.dma_start(wp[:], wp_d[:, :])

        # tables + cams on other queues (vector/scalar/tensor trigger)
        csc = ptab.tile([128, L], f32, tag="tab", bufs=3, name="csc")
        css2 = ptab.tile([128, L], f32, tag="tab", bufs=3, name="css2")
        css = ptab.tile([128, L], f32, tag="tab", bufs=3, name="css")
        nc.scalar.dma_start(csc[:], csc_d[:, :])
        nc.scalar.dma_start(css2[:], css2_d[:, :])
        nc.scalar.dma_start(css[:], css_d[:, :])
        cam = {}
        for nm, dten in (("camq", camq_d), ("camq2", camq2_d),
                         ("camk", camk_d), ("camk2", camk2_d),
                         ("camo", camo_d), ("camo2", camo2_d)):
            t = pcam.tile([128, 8 * 128], f32r, tag="cam", bufs=6, name=nm)
            nc.scalar.dma_start(t[:], dten[:, :].bitcast(f32r))
            cam[nm] = t
        onescol = ptab.tile([128, 4], bf16, tag="ones", bufs=1, name="ones")
        nc.scalar.dma_start(onescol[:], ones_d[:, :])

        qp = [None, None]
        kp = [None, None]
        vt = [[None] * 16, [None] * 16]
        opT = [[None] * 4, [None] * 4]

        def proj_phase(pt):
            """Projections + PRoPE for head pair pt."""
            with tc.tile_pool(name=f"pe12_{pt}", bufs=6) as pe12, \
                 tc.tile_pool(name=f"psP_{pt}", bufs=3, space="PSUM") as psP, \
                 tc.tile_pool(name=f"psC_{pt}", bufs=2, space="PSUM") as psC:
                qp[pt] = pqk.tile([128, L], bf16, tag="qk", bufs=4, name=f"qp{pt}")
                kp[pt] = pqk.tile([128, L], bf16, tag="qk", bufs=4, name=f"kp{pt}")
                for t in range(16):
                    vt[pt][t] = pvt.tile([128, 130], bf16, tag="vt", bufs=32,
                                         name=f"vt{pt}_{t}")
                    nc.vector.tensor_copy(vt[pt][t][:, 64:65], onescol[:, 0:1])
                    nc.vector.tensor_copy(vt[pt][t][:, 129:130], onescol[:, 0:1])
                wsl = lambda d: slice(256 * d + 128 * pt, 256 * d + 128 * pt + 128)
                for lb in range(4):
                    lsl = slice(512 * lb, 512 * lb + 512)
                    for (w8, dest, cA, cB) in ((wq, qp[pt], cam["camq"], cam["camq2"]),
                                               (wk, kp[pt], cam["camk"], cam["camk2"])):
                        acc = psP.tile([128, 512], f32, tag="acc", bufs=3)
                        for d in range(8):
                            nc.tensor.matmul(acc[:], w8[:, wsl(d)],
                                             xt[:, xcol(d, lb):xcol(d, lb) + 512],
                                             start=(d == 0), stop=(d == 7))
                        e1 = pe12.tile([128, 512], f32r, tag="e", bufs=6)
                        e2 = pe12.tile([128, 512], f32r, tag="e", bufs=6)
                        nc.vector.tensor_tensor(e1[:], csc[:, lsl].bitcast(f32r),
                                                acc[:].bitcast(f32r), op=ALU.mult)
                        nc.vector.tensor_tensor(e2[:], css2[:, lsl].bitcast(f32r),
                                                acc[:].bitcast(f32r), op=ALU.mult)
                        cc = psC.tile([128, 512], f32, tag="cc", bufs=2)
                        for ci in range(2):
                            c = 2 * lb + ci
                            csl = slice(256 * ci, 256 * ci + 256)
                            nc.tensor.matmul(cc[:, csl],
                                             cA[:, 128 * c:128 * c + 128],
                                             e1[:, csl], start=True, stop=False)
                            nc.tensor.matmul(cc[:, csl],
                                             cB[:, 128 * c:128 * c + 128],
                                             e2[:, csl], start=False, stop=True)
                        nc.scalar.copy(dest[:, lsl], cc[:])
                    # v chain: right-multiply into (token, dim) layout
                    acc = psP.tile([128, 512], f32, tag="acc", bufs=3)
                    for d in range(8):
                        nc.tensor.matmul(acc[:], wv[:, wsl(d)],
                                         xt[:, xcol(d, lb):xcol(d, lb) + 512],
                                         start=(d == 0), stop=(d == 7))
                    e1 = pe12.tile([128, 512], f32r, tag="e", bufs=6)
                    e2 = pe12.tile([128, 512], f32r, tag="e", bufs=6)
                    nc.vector.tensor_tensor(e1[:], csc[:, lsl].bitcast(f32r),
                                            acc[:].bitcast(f32r), op=ALU.mult)
                    nc.vector.tensor_tensor(e2[:], css2[:, lsl].bitcast(f32r),
                                            acc[:].bitcast(f32r), op=ALU.mult)
                    for ti in range(4):
                        t = 4 * lb + ti
                        c = 2 * lb + ti // 2
                        tsl = slice(128 * ti, 128 * ti + 128)
                        vo = psC.tile([128, 128], f32, tag="vo", bufs=2)
                        nc.tensor.matmul(vo[:], e1[:, tsl],
                                         cam["camk"][:, 128 * c:128 * c + 128],
                                         start=True, stop=False)
                        nc.tensor.matmul(vo[:], e2[:, tsl],
                                         cam["camk2"][:, 128 * c:128 * c + 128],
                                         start=False, stop=True)
                        nc.scalar.copy(vt[pt][t][:, 0:64], vo[:, 0:64])
                        nc.scalar.copy(vt[pt][t][:, 65:129], vo[:, 64:128])

        def attn_phase(pt):
            with tc.tile_pool(name=f"pat_{pt}", bufs=6) as pat, \
                 tc.tile_pool(name=f"psm_{pt}", bufs=10) as psm, \
                 tc.tile_pool(name=f"psS_{pt}", bufs=2, space="PSUM") as psS, \
                 tc.tile_pool(name=f"psO_{pt}", bufs=4, space="PSUM") as psO:
                for qg in range(2):
                    po = {}
                    for hi in range(2):
                        for qh in range(2):
                            po[hi, qh] = psO.tile(
                                [128, 512], f32, tag="po", bufs=4,
                                name=f"po{pt}_{qg}_{hi}_{qh}")
                    for t in range(16):
                        ksl = slice(128 * t, 128 * t + 128)
                        at2 = [None, None]
                        pss = [psS.tile([128, 1024], f32, tag="sc", bufs=2,
                                        name=f"sc{pt}_{qg}_{t}_{hi}")
                               for hi in range(2)]
                        for qh in range(2):
                            qsl = slice(1024 * qg + 512 * qh,
                                        1024 * qg + 512 * qh + 512)
                            for hi in range(2):
                                hsl = slice(64 * hi, 64 * hi + 64)
                                nc.tensor.matmul(
                                    pss[hi][:, 512 * qh:512 * qh + 512],
                                    kp[pt][hsl, ksl], qp[pt][hsl, qsl],
                                    start=True, stop=True,
                                    tile_position=(64 * hi, 0))
                        for hi in range(2):
                            at = pat.tile([128, 1024], bf16, tag="at", bufs=6,
                                          name=f"at{pt}_{qg}_{t}_{hi}")
                            nc.scalar.activation(at[:], pss[hi][:], ACT.Exp,
                                                 scale=0.125)
                            at2[hi] = at
                        for hi in range(2):
                            for qh in range(2):
                                nc.tensor.matmul(
                                    po[hi, qh][0:65, :],
                                    vt[pt][t][:, 65 * hi:65 * hi + 65],
                                    at2[hi][:, 512 * qh:512 * qh + 512],
                                    start=(t == 0), stop=(t == 15))
                    # early po evacuation: frees the 4 PSUM banks for the
                    # next qg's accumulators without waiting on the bounce
                    ocr = {}
                    dnr = {}
                    for qh in range(2):
                        r = psm.tile([128, 512], f32r, tag="ocr", bufs=2,
                                     name=f"ocr{pt}_{qg}_{qh}")
                        for hi in range(2):
                            nc.vector.tensor_copy(r[64 * hi:64 * hi + 64, :],
                                                  po[hi, qh][0:64, :])
                            dn = psm.tile([1, 512], f32, tag="dn", bufs=4,
                                          name=f"dn{pt}_{qg}_{hi}_{qh}")
                            nc.vector.tensor_copy(dn[:], po[hi, qh][64:65, :])
                            dnr[hi, qh] = dn
                        ocr[qh] = r
                    for qh in range(2):
                        qb = 2 * qg + qh
                        qsl = slice(512 * qb, 512 * qb + 512)
                        rd = psm.tile([128, 512], f32, tag="rd", bufs=2,
                                      name=f"rd{pt}_{qb}")
                        for hi in range(2):
                            bi = (pt * 4 + qb) * 2 + hi
                            nc.sync.dma_start(bounce_d[bi, :][None, :],
                                              dnr[hi, qh][:])
                            rc = psm.tile([128, 4], f32, tag="rc", bufs=2,
                                          name=f"rc{pt}_{qb}_{hi}")
                            nc.sync.dma_start(
                                rc[:], bounce_d[bi:bi + 1, :].rearrange(
                                    "a (p f) -> (a p) f", p=128))
                            rc2 = psm.tile([128, 4], f32, tag="rc2", bufs=2,
                                           name=f"rc2{pt}_{qb}_{hi}")
                            nc.vector.reciprocal(rc2[:], rc[:])
                            nc.sync.dma_start(bounce2_d[bi, :][None, :].rearrange(
                                "a (p f) -> (a p) f", p=128), rc2[:])
                            nc.sync.dma_start(
                                rd[64 * hi:64 * hi + 64, :],
                                bounce2_d[bi, :][None, :].to_broadcast((64, 512)))
                        # m1/m2 read the unscaled ocr directly: the 1/dn
                        # scale commutes past the camera matmul and the
                        # D-rotation (per-head dn is constant within each
                        # head's row block), so rd applies at the very end
                        # and the bounce chain never gates the PSUM ring.
                        m1 = psO.tile([128, 512], f32, tag="po", bufs=4,
                                      name=f"m1{pt}_{qb}")
                        m2 = psO.tile([128, 512], f32, tag="po", bufs=4,
                                      name=f"m2{pt}_{qb}")
                        for ci in range(2):
                            c = 2 * qb + ci
                            csl = slice(256 * ci, 256 * ci + 256)
                            nc.tensor.matmul(m1[:, csl],
                                             cam["camo"][:, 128 * c:128 * c + 128],
                                             ocr[qh][:, csl],
                                             start=True, stop=True)
                            nc.tensor.matmul(m2[:, csl],
                                             cam["camo2"][:, 128 * c:128 * c + 128],
                                             ocr[qh][:, csl],
                                             start=True, stop=True)
                        ta = psm.tile([128, 512], f32, tag="ta", bufs=2)
                        tb = psm.tile([128, 512], f32, tag="tb", bufs=2)
                        nc.vector.tensor_tensor(ta[:], csc[:, qsl], m1[:], op=ALU.mult)
                        nc.vector.tensor_tensor(tb[:], css[:, qsl], m2[:], op=ALU.mult)
                        tc_ = psm.tile([128, 512], f32, tag="tc", bufs=2)
                        nc.vector.tensor_tensor(tc_[:], ta[:], tb[:],
                                                op=ALU.subtract)
                        opT[pt][qb] = popt.tile([128, 512], bf16, tag="opt", bufs=8,
                                                name=f"opT{pt}_{qb}")
                        nc.vector.tensor_tensor(opT[pt][qb][:], tc_[:], rd[:],
                                                op=ALU.mult)
                if pt == 1:
                    # output projection: reuse freed po slots, overlap the
                    # last qb's bounce chain and spread yp DMAs
                    for qb in range(4):
                        for li in range(4):
                            lt = 4 * qb + li
                            tsl = slice(128 * li, 128 * li + 128)
                            yo = py.tile([128, DM], f32, tag="yo", bufs=4)
                            for nb in range(2):
                                nsl = slice(512 * nb, 512 * nb + 512)
                                ys = psO.tile([128, 512], f32, tag="po", bufs=4)
                                nc.tensor.matmul(ys[:], opT[0][qb][:, tsl],
                                                 wp[:, nsl],
                                                 start=True, stop=False)
                                nc.tensor.matmul(ys[:], opT[1][qb][:, tsl],
                                                 wp[:, DM + nsl.start:
                                                    DM + nsl.stop],
                                                 start=False, stop=True)
                                nc.scalar.copy(yo[:, nsl], ys[:])
                            nc.sync.dma_start(
                                yp_d[128 * lt:128 * lt + 128, :], yo[:])

        proj_phase(0)
        attn_phase(0)
        proj_phase(1)
        attn_phase(1)

    return nc


def _split_multi_waits(nc):
    """This walrus build accepts only one sync-wait per instruction; move
    extras onto standalone InstEventSemaphore ops just before."""
    import concourse.mybir as mybir
    n = 0
    for f in nc.m.functions:
        for bb in f.blocks:
            new_insts = []
            for inst in bb.instructions:
                si = inst.sync_info
                if si is not None and si.on_wait and len(si.on_wait) > 1:
                    waits = list(si.on_wait)
                    for w in waits[:-1]:
                        n += 1
                        new_insts.append(mybir.InstEventSemaphore(
                            name=f"I-splitw-{n}", engine=inst.engine,
                            ins=[], outs=[],
                            sync_info=mybir.SyncInfo(on_wait=[w], on_update=[]),
                        ))
                    inst.sync_info = mybir.SyncInfo(
                        on_wait=[waits[-1]], on_update=list(si.on_update or []))
                new_insts.append(inst)
            bb.instructions = new_insts
    return n


def make_in_maps(x, viewmats, Ks, w_qkv, w_proj):
    import ml_dtypes
    bft = ml_dtypes.bfloat16
    x = np.asarray(x, np.float32)
    viewmats = np.asarray(viewmats, np.float32)
    Ks = np.asarray(Ks, np.float32)
    w_qkv = np.asarray(w_qkv, np.float32)
    w_proj = np.asarray(w_proj, np.float32)

    csc, css2, css = _rope_tables()
    P, P_inv = _cam_mats(viewmats, Ks)
    w3 = w_qkv.reshape(3, H, DH, DM)
    I32 = np.eye(32, dtype=np.float32)
    perm = np.arange(128) ^ 1          # pair swap

    def pack_w(wT):                     # (DM, C) -> (128, 8*C)
        C = wT.shape[1]
        return np.ascontiguousarray(
            wT.reshape(8, 128, C).transpose(1, 0, 2).reshape(128, 8 * C))

    in_maps = []
    for core in range(N_CORES):
        b, hg = divmod(core, HPG)
        heads = slice(4 * hg, 4 * hg + 4)
        xT = np.ascontiguousarray(x[b].T)                        # (DM, L)
        wqT = w3[0, heads].reshape(HD4, DM).T                    # (DM, 256)
        wkT = w3[1, heads].reshape(HD4, DM).T
        wvT = w3[2, heads].reshape(HD4, DM).T
        wpT = w_proj[:, 256 * hg:256 * hg + 256].T               # (256, DM)

        camq = np.stack([np.kron(I32, P_inv[b, c]) for c in range(CAMS)])
        camk = np.stack([np.kron(I32, P[b, c].T) for c in range(CAMS)])
        camo = np.stack([np.kron(I32, P_inv[b, c].T) for c in range(CAMS)])
        camq2 = camq[:, perm, :]        # row pair-swap (lhsT B variant)
        camk2 = camk[:, perm, :]
        camo2 = camo[:, :, perm]        # col pair-swap (output-side variant)

        def pack_cam(cm):               # (8,128,128) -> (128, 8*128)
            return np.ascontiguousarray(
                cm.transpose(1, 0, 2).reshape(128, 8 * 128)).astype(np.float32)

        xt_p = (xT.reshape(8, 128, 4, 512).transpose(1, 2, 0, 3)
                .reshape(128, 8 * L))  # [p, 4096*lb + 512*d + f]
        in_maps.append({
            "xt": np.ascontiguousarray(xt_p).astype(bft),
            "wq": pack_w(wqT).astype(bft),
            "wk": pack_w(wkT).astype(bft),
            "wv": pack_w(wvT).astype(bft),
            "wp": np.ascontiguousarray(
                wpT.reshape(2, 128, DM).transpose(1, 0, 2).reshape(128, 2 * DM)
            ).astype(bft),
            "csc": csc, "css2": css2, "css": css,
            "camq": pack_cam(camq), "camq2": pack_cam(camq2),
            "camk": pack_cam(camk), "camk2": pack_cam(camk2),
            "camo": pack_cam(camo), "camo2": pack_cam(camo2),
            "ones": np.ones((128, 4), bft),
        })
    return in_maps


last_results = None


def kernel(x, viewmats, Ks, w_qkv, w_proj):
    from concourse.bass_utils import run_bass_kernel_spmd
    global last_results
    nc = _build_nc()
    if not getattr(nc, "_waits_split", False):
        _split_multi_waits(nc)
        nc._waits_split = True
    in_maps = make_in_maps(x, viewmats, Ks, w_qkv, w_proj)
    res = run_bass_kernel_spmd(nc, in_maps, core_ids=list(range(N_CORES)))
    last_results = res
    outs = res.results
    y = np.zeros((B, L, DM), np.float32)
    for core in range(N_CORES):
        b = core // HPG
        y[b] += outs[core]["yp"]
    return y

